# revision 7
# baseline (speedup 1.0000x reference)
"""Trainium2 Bass kernel: 2-layer LIF SNN (DelayedXOR vanilla SNN).

Reference semantics (per timestep t, fp32):
    h1 = x_t @ W1.T + b1
    v1 = v1 + (h1 - v1)/2 ;  s1 = (v1 >= 1) ;  v1 = v1 * (1 - s1)
    h2 = s1 @ W2.T + b2
    v2 = v2 + (h2 - v2)/2 ;  s2 = (v2 >= 1) ;  v2 = v2 * (1 - s2)
    out = sum_{t >= T/2} s2                       # [B, O]

Kernel strategy (per core, batch-sharded 128 -> 16, weights replicated,
no collectives):
  * Fold the 1/2 decay into the weights (exact: powers of two).  Track
    u_t = pre-reset potential with the reset folded into the next step:
        u_t = 0.5 * u_{t-1} * (u_{t-1} < 1) + h_t      (h = 0.5*(x@W1.T+b1))
    This is ONE custom DVE op per step (registered at import time):
        out = (Src0 * (Src0 < C0)) * C1 + Src1
  * Layer-1 matmuls have no recurrence: computed on the PE in groups of
    G=8 steps directly into PSUM; the DVE op reads PSUM as in1.
  * Spikes s = (u >= 1) computed on GpSimd (keeps the DVE backbone free).
  * Layer-2 matmul per group over the spike tile (8 accumulating chunk
    matmuls, contraction over H=1024); bias applied by ScalarE on the
    PSUM->SBUF copy.  Layer-2 LIF work lags 2 groups so the DVE never
    stalls on the PE/ACT pipeline.
  * Output spike counts accumulated in PSUM via identity matmuls.

Layouts per core (BL = 16 batch):
  u1 state     [128p, (c8, b16)]   hidden h = c*128+p
  h1 psum      [128p, c8, (t8, b16)]
  s1 group     [128p, (t8, c8, b16)]
  L2 out psum  [128o, (t8, b16)]
  u2 state     [128o, b16]
"""

import os
import sys
import tempfile

for _p in ("/opt/trn_rl_repo",):
    if _p not in sys.path:
        sys.path.insert(0, _p)

import numpy as np

B, T, I, H, O = 128, 2048, 128, 1024, 128
NCORES = 8
BL = B // NCORES          # 16 batch per core
G = 8                     # timesteps per group
NCH = H // 128            # 8 hidden chunks
V2_LAG = 2                # groups of lag for layer-2 LIF processing

_prog_cache = {}
_LIF_OP = None


def _register_lif_op():
    """Register the fused LIF-step custom DVE op (idempotent)."""
    global _LIF_OP
    if _LIF_OP is not None:
        return _LIF_OP
    import concourse.dve_ops as dve_ops
    from concourse.dve_spec import Spec, Src0, Src1, C0, C1, lower
    from concourse.dve_uop import DveOpSpec

    name = "LIF_STEP_ANT"
    for o in dve_ops.OPS:
        if o.name == name:
            _LIF_OP = o
            return o

    def ref(in0, in1, s0, s1, imm2):
        w = (in0 * (in0 < s0)).astype(np.float32)
        return (w * np.float32(s1) + in1.reshape(in0.shape)).astype(np.float32)

    spec = Spec(body=(Src0 * (Src0 < C0)) * C1 + Src1, reference=ref)
    op = dve_ops.DveOp(name, spec, subdim=False, uops_sha={})
    dve_ops.OPS.append(op)
    dve_ops.CUSTOM_DVE_SPECS[name] = spec
    dve_ops._SUB_OPCODE_FOR_NAME[name] = (
        dve_ops._CUSTOM_DVE_ROW_BASE + len(dve_ops.OPS) - 1
    )
    opcode = dve_ops.get_dve_sub_opcode(name)
    for ver in ("v3", "v4"):
        tmp = DveOpSpec(
            name=name, opcode=opcode, uops=lower(spec, ver=ver), rd1_en=True
        )
        op.uops_sha[ver] = tmp.sha(ver)
    _LIF_OP = op
    return op


def build_program(t_steps=T):
    """Builds the single-core Bass/Tile program (identical on all cores)."""
    from contextlib import ExitStack

    import concourse.bass as bass
    import concourse.tile as tile
    from concourse import bacc, mybir

    lif = _register_lif_op()

    f32 = mybir.dt.float32
    Alu = mybir.AluOpType
    Act = mybir.ActivationFunctionType

    ng = t_steps // G
    dec_g = ng // 2          # groups >= dec_g contribute to the output sum

    nc = bacc.Bacc("TRN2", target_bir_lowering=False, debug=False)

    xT_d = nc.dram_tensor("xT", [128, t_steps * BL], f32, kind="ExternalInput")
    w1t_d = nc.dram_tensor("w1t", [128, H], f32, kind="ExternalInput")
    w2st_d = nc.dram_tensor("w2st", [H, 128], f32, kind="ExternalInput")
    b1k_d = nc.dram_tensor("b1k", [NCH, 128], f32, kind="ExternalInput")
    sel8_d = nc.dram_tensor("sel8", [NCH, NCH * G * BL], f32, kind="ExternalInput")
    b2s_d = nc.dram_tensor("b2s", [128, 1], f32, kind="ExternalInput")
    eye_d = nc.dram_tensor("eye", [128, 128], f32, kind="ExternalInput")
    out_d = nc.dram_tensor("outT", [128, BL], f32, kind="ExternalOutput")

    GB = G * BL  # columns per group (t, b) = 128

    with ExitStack() as ctx:
        tc = ctx.enter_context(tile.TileContext(nc))
        const = ctx.enter_context(tc.tile_pool(name="const", bufs=1))
        state = ctx.enter_context(tc.tile_pool(name="state", bufs=1))
        xpool = ctx.enter_context(tc.tile_pool(name="xin", bufs=4))
        s1pool = ctx.enter_context(tc.tile_pool(name="s1g", bufs=2))
        s2pool = ctx.enter_context(tc.tile_pool(name="s2g", bufs=2))
        h2pool = ctx.enter_context(tc.tile_pool(name="h2g", bufs=4))
        ph1 = ctx.enter_context(
            tc.tile_pool(name="ph1", bufs=2, space=bass.MemorySpace.PSUM)
        )
        pg = ctx.enter_context(
            tc.tile_pool(name="pg", bufs=2, space=bass.MemorySpace.PSUM)
        )
        pacc = ctx.enter_context(
            tc.tile_pool(name="pacc", bufs=1, space=bass.MemorySpace.PSUM)
        )

        # ---- constants ----
        w1t = const.tile([128, H], f32)
        nc.sync.dma_start(w1t[:], w1t_d[:])
        # w2st sbuf layout [p, c*128+o] <- dram [c*128+p, o]
        w2st = const.tile([128, NCH * 128], f32)
        nc.sync.dma_start(
            w2st[:].rearrange("p (c o) -> p c o", c=NCH),
            w2st_d[:].rearrange("(c p) o -> p c o", c=NCH),
        )
        b1k = const.tile([NCH, 128], f32)
        nc.sync.dma_start(b1k[:], b1k_d[:])
        sel8 = const.tile([NCH, NCH * G * BL], f32)
        nc.sync.dma_start(sel8[:], sel8_d[:])
        b2s = const.tile([128, 1], f32)
        nc.sync.dma_start(b2s[:], b2s_d[:])
        eye = const.tile([128, 128], f32)
        nc.sync.dma_start(eye[:], eye_d[:])

        # ---- state (ping-pong pairs) ----
        u1 = [state.tile([128, NCH * BL], f32, name=f"u1_{i}") for i in range(2)]
        u2 = [state.tile([128, BL], f32, name=f"u2_{i}") for i in range(2)]
        out_sb = state.tile([128, BL], f32)
        nc.vector.memset(u1[0][:], 0.0)
        nc.vector.memset(u2[0][:], 0.0)

        acc = pacc.tile([128, BL], f32)

        pending = []  # deferred layer-2 LIF work: (h2 tile, group index)

        def emit_v2(h2g, gprev):
            s2g = s2pool.tile([128, GB], f32)
            for tau in range(G):
                sl = slice(tau * BL, (tau + 1) * BL)
                cur, nxt = u2[tau % 2], u2[(tau + 1) % 2]
                # u2' = 0.5*u2*(u2<1) + h2s_t
                nc.vector._custom_dve(
                    lif, out=nxt[:], in0=cur[:], in1=h2g[:, sl], s0=1.0, s1=0.5
                )
                # s2_t = (u2' >= 1)
                nc.gpsimd.tensor_scalar(s2g[:, sl], nxt[:], 1.0, None, Alu.is_ge)
            if gprev >= dec_g:
                first = gprev == dec_g
                last = gprev == ng - 1
                for tau in range(G):
                    sl = slice(tau * BL, (tau + 1) * BL)
                    nc.tensor.matmul(
                        acc[:],
                        eye[:],
                        s2g[:, sl],
                        start=(first and tau == 0),
                        stop=(last and tau == G - 1),
                        skip_group_check=True,
                    )

        for g in range(ng):
            # ---- input tile for this group ----
            xt = xpool.tile([128, GB], f32)
            nc.sync.dma_start(xt[:], xT_d[:, g * GB : (g + 1) * GB])

            # ---- phase A: h1 for the group's 8 steps ----
            h1p = ph1.tile([128, NCH, GB], f32)
            # A PSUM zero-region is one 2KB bank (4 chunk slices): start=True
            # only on the first matmul touching each bank.
            for c in range(NCH):
                nc.tensor.matmul(
                    h1p[:, c, :],
                    w1t[:, c * 128 : (c + 1) * 128],
                    xt[:],
                    start=(c % 4 == 0),
                    stop=False,
                    skip_group_check=True,
                )
            # bias: h1p[p, c, :] += 0.5*b1[c*128+p]  (K=8 selector matmul)
            half = NCH * GB // 2
            for piece in range(2):
                sl = slice(piece * half, (piece + 1) * half)
                nc.tensor.matmul(
                    h1p[:].rearrange("p c n -> p (c n)")[:, sl],
                    b1k[:],
                    sel8[:, sl],
                    start=False,
                    stop=True,
                    skip_group_check=True,
                )

            # ---- layer-1 LIF, one fused DVE op per step ----
            s1g = s1pool.tile([128, G * 128], f32)  # [p, (t, c, b)]
            for tau in range(G):
                cur, nxt = u1[tau % 2], u1[(tau + 1) % 2]
                nc.vector._custom_dve(
                    lif,
                    out=nxt[:],
                    in0=cur[:],
                    in1=h1p[:, :, tau * BL : (tau + 1) * BL],
                    s0=1.0,
                    s1=0.5,
                )
                nc.gpsimd.tensor_scalar(
                    s1g[:, tau * 128 : (tau + 1) * 128], nxt[:], 1.0, None, Alu.is_ge
                )

            # ---- layer-2 matmul for the group ----
            pgt = pg.tile([128, GB], f32)  # [o, (t, b)]
            pgv = pgt[:].rearrange("o (t b) -> o t b", t=G)
            s1v = s1g[:].rearrange("p (t c b) -> p t c b", t=G, c=NCH)
            for c in range(NCH):
                nc.tensor.matmul(
                    pgv,
                    w2st[:, c * 128 : (c + 1) * 128],
                    s1v[:, :, c, :],
                    start=(c == 0),
                    stop=(c == NCH - 1),
                    skip_group_check=True,
                )
            # h2s = psum + 0.5*b2 (per-partition bias)
            h2g = h2pool.tile([128, GB], f32)
            nc.scalar.activation(h2g[:], pgt[:], Act.Identity, bias=b2s[:], scale=1.0)

            # ---- deferred layer-2 LIF (lags V2_LAG groups) ----
            pending.append((h2g, g))
            if len(pending) > V2_LAG:
                emit_v2(*pending.pop(0))

        for item in pending:
            emit_v2(*item)

        # ---- output: acc holds sum of s2 over the decision window ----
        nc.vector.tensor_copy(out_sb[:], acc[:])
        nc.sync.dma_start(out_d[:], out_sb[:])

    nc.compile()
    return nc


def make_core_inputs(x, W1, b1, W2, b2, t_steps=T):
    """Host-side shard + layout prep. Returns one input map per core."""
    x = np.ascontiguousarray(x, dtype=np.float32)
    W1 = np.asarray(W1, dtype=np.float32)
    b1 = np.asarray(b1, dtype=np.float32)
    W2 = np.asarray(W2, dtype=np.float32)
    b2 = np.asarray(b2, dtype=np.float32)

    w1t = np.ascontiguousarray((0.5 * W1).T)              # [I, H]
    w2st = np.ascontiguousarray((0.5 * W2).T)             # [H, O]
    b1k = np.ascontiguousarray((0.5 * b1).reshape(NCH, 128))
    sel8 = np.kron(np.eye(NCH, dtype=np.float32), np.ones((1, G * BL), np.float32))
    sel8 = np.ascontiguousarray(sel8)                     # [8, 8*128]
    b2s = np.ascontiguousarray((0.5 * b2).reshape(128, 1))
    eye = np.eye(128, dtype=np.float32)

    ins = []
    for core in range(NCORES):
        xs = x[core * BL : (core + 1) * BL, :t_steps, :]  # [BL, t, I]
        xT = np.ascontiguousarray(xs.transpose(2, 1, 0).reshape(128, t_steps * BL))
        ins.append(
            {
                "xT": xT,
                "w1t": w1t,
                "w2st": w2st,
                "b1k": b1k,
                "sel8": sel8,
                "b2s": b2s,
                "eye": eye,
            }
        )
    return ins


def _install_ntff_hook():
    """Provide the antenv.axon_hooks shim if the image lacks it (needed only
    for trace=True profiling under axon)."""
    import types

    try:
        from antenv.axon_hooks import get_axon_ntff_profile_hook  # noqa: F401

        return
    except ImportError:
        pass
    import antenv
    from trn_agent_boot.trn_boot import _ntff_profile_via_ctypes

    mod = types.ModuleType("antenv.axon_hooks")
    box = {"h": None}
    mod.set_axon_ntff_profile_hook = lambda h: box.__setitem__("h", h)
    mod.get_axon_ntff_profile_hook = lambda: box["h"]
    sys.modules["antenv.axon_hooks"] = mod
    antenv.axon_hooks = mod
    so = "/opt/axon/libaxon_pjrt.so"
    if os.path.exists(so):
        mod.set_axon_ntff_profile_hook(_ntff_profile_via_ctypes(so))


def run(x, W1, b1, W2, b2, t_steps=T, trace=False):
    from concourse.bass_utils import run_bass_kernel_spmd

    if trace:
        _install_ntff_hook()

    if t_steps not in _prog_cache:
        _prog_cache[t_steps] = build_program(t_steps)
    nc = _prog_cache[t_steps]

    ins = make_core_inputs(x, W1, b1, W2, b2, t_steps)
    res = run_bass_kernel_spmd(
        nc, ins, list(range(NCORES)), trace=trace, tmpdir=tempfile.mkdtemp()
    )
    out = np.empty((B, O), dtype=np.float32)
    for core in range(NCORES):
        out[core * BL : (core + 1) * BL, :] = res.results[core]["outT"].T
    return out, res


def kernel(x, W1, b1, W2, b2):
    out, _ = run(x, W1, b1, W2, b2)
    return out
